# revision 1
# baseline (speedup 1.0000x reference)
"""GRU decoder kernel for 8 trn2 NeuronCores.

Algorithm notes (derivation from the reference GruDecoder):
  x_{t+1} = y_t = h_{t+1} @ W_fc.T + b_fc, so the input-path matmul folds into
  the recurrence:  gi_t = h_t @ (W_ih @ W_fc).T + (b_ih + W_ih @ b_fc)  (t>=1).
  r/z gates use gi+gh, so those rows of the folded matrix and W_hh are summed
  host-side; the n-gate keeps gi_n / gh_n separate (r multiplies only gh_n).
  Per step this leaves ONE [B,1024] @ [1024, 4*1024] matmul + elementwise.

Sharding: model-parallel over the hidden dim. Core k owns hidden slice
  J_k = [128k, 128k+128): it computes r/z/n/h_new for those 128 hidden dims
  for the FULL batch of 256 (so the PE streams N=256 per weight tile), then an
  AllGather rebuilds the full h_{t+1}^T [1024, 256] on every core. The output
  projection y_t = h_{t+1} @ W_fc.T + b_fc is computed from the gathered h
  with core k owning output columns [96k, 96k+96).
"""

import os
import sys

sys.path.insert(0, "/opt/trn_rl_repo")

import numpy as np

H = 1024
OUT = 768
B = 256
T = int(os.environ.get("GRU_T", "256"))
NCORES = 8
MSLICE = 4 * 128  # per-core folded gate rows (r,z,ni,nh) x 128 hidden dims
OSLICE = OUT // NCORES  # 96 output cols per core
K_REC = H // 128  # 8 K-tiles for the recurrence matmul
K_0 = (OUT + H) // 128  # 14 K-tiles for the step-0 matmul ([x0; h0])

_cache = {}


def _build_program():
    import concourse.mybir as mybir
    from concourse import bacc, tile

    dt = mybir.dt
    AF = mybir.ActivationFunctionType
    RG = [list(range(NCORES))]

    nc = bacc.Bacc(num_devices=NCORES)

    w_rec_d = nc.dram_tensor("w_rec", [128, K_REC, MSLICE], dt.bfloat16, kind="ExternalInput")
    w0_d = nc.dram_tensor("w0", [128, K_0, MSLICE], dt.bfloat16, kind="ExternalInput")
    wfc_d = nc.dram_tensor("wfc", [128, K_REC, OSLICE], dt.bfloat16, kind="ExternalInput")
    rhs0_d = nc.dram_tensor("rhs0", [128, K_0, B], dt.bfloat16, kind="ExternalInput")
    h0own_d = nc.dram_tensor("h0own", [128, B], dt.bfloat16, kind="ExternalInput")
    biasS_d = nc.dram_tensor("biasS", [128, 4], dt.float32, kind="ExternalInput")
    bias0_d = nc.dram_tensor("bias0", [128, 4], dt.float32, kind="ExternalInput")
    bfc_d = nc.dram_tensor("bfc", [OSLICE, 1], dt.float32, kind="ExternalInput")
    out_d = nc.dram_tensor("out", [T, OSLICE, B], dt.float32, kind="ExternalOutput")

    with tile.TileContext(nc) as tc:
        with (
            tc.tile_pool(name="wp", bufs=1) as wp,
            tc.tile_pool(name="hp", bufs=3) as hp,
            tc.tile_pool(name="ep", bufs=2) as ep,
            tc.tile_pool(name="pp", bufs=1, space="PSUM") as pp,
            tc.tile_pool(name="yp", bufs=2, space="PSUM") as yp,
            tc.tile_pool(name="dp", bufs=2, space="DRAM") as dp,
        ):
            wrec_sb = wp.tile([128, K_REC, MSLICE], dt.bfloat16)
            nc.sync.dma_start(wrec_sb[:], w_rec_d[:])
            w0_sb = wp.tile([128, K_0, MSLICE], dt.bfloat16)
            nc.sync.dma_start(w0_sb[:], w0_d[:])
            wfc_sb = wp.tile([128, K_REC, OSLICE], dt.bfloat16)
            nc.sync.dma_start(wfc_sb[:], wfc_d[:])
            rhs0_sb = wp.tile([128, K_0, B], dt.bfloat16)
            nc.sync.dma_start(rhs0_sb[:], rhs0_d[:])
            biasS_sb = wp.tile([128, 4], dt.float32)
            nc.sync.dma_start(biasS_sb[:], biasS_d[:])
            bias0_sb = wp.tile([128, 4], dt.float32)
            nc.sync.dma_start(bias0_sb[:], bias0_d[:])
            bfc_sb = wp.tile([OSLICE, 1], dt.float32)
            nc.sync.dma_start(bfc_sb[:], bfc_d[:])

            CH = 2
            Bc = B // CH  # 128 batch columns per chunk
            h_bf = []
            for c in range(CH):
                hb = hp.tile([128, Bc], dt.bfloat16, tag=f"hs{c}")
                nc.sync.dma_start(hb[:], h0own_d[:, c * Bc : (c + 1) * Bc])
                h_bf.append(hb)

            # Two-chunk software pipeline: while chunk 0 is in its
            # elem -> DMA -> AllGather -> DMA chain, chunk 1 owns the PE
            # (and vice versa), so the per-step serial latency is hidden.
            hall = [None, None]
            for t in range(T):
                for c in range(CH):
                    col = slice(c * Bc, (c + 1) * Bc)
                    if t == 0:
                        nk, lhs, bias = K_0, w0_sb, bias0_sb
                        rhs_of = lambda kt, _c=c: rhs0_sb[:, kt, _c * Bc : (_c + 1) * Bc]
                    else:
                        nk, lhs, bias = K_REC, wrec_sb, biasS_sb
                        rhs_of = lambda kt, _h=hall[c]: _h[kt // 4][:, kt % 4, :]

                    # one PSUM bank holds all 4 gate blocks for this chunk
                    P = pp.tile([128, 4 * Bc], dt.float32, tag=f"pg{c}")
                    for m in (0, 3, 2, 1):
                        for kt in range(nk):
                            nc.tensor.matmul(
                                P[:, m * Bc : (m + 1) * Bc],
                                lhs[:, kt, m * 128 : (m + 1) * 128],
                                rhs_of(kt),
                                start=(kt == 0),
                                stop=(kt == nk - 1),
                            )
                    Pr = P[:, 0:Bc]
                    Pz = P[:, Bc : 2 * Bc]
                    Pni = P[:, 2 * Bc : 3 * Bc]
                    Pnh = P[:, 3 * Bc : 4 * Bc]

                    r = ep.tile([128, Bc], dt.float32, tag=f"r{c}")
                    nc.scalar.activation(r[:], Pr, AF.Sigmoid, bias=bias[:, 0:1])
                    z = ep.tile([128, Bc], dt.float32, tag=f"z{c}")
                    nc.scalar.activation(z[:], Pz, AF.Sigmoid, bias=bias[:, 1:2])
                    t2 = ep.tile([128, Bc], dt.float32, tag=f"t2{c}")
                    nc.vector.scalar_tensor_tensor(
                        t2[:], Pnh, bias[:, 3:4], r[:],
                        mybir.AluOpType.add, mybir.AluOpType.mult,
                    )
                    t3 = ep.tile([128, Bc], dt.float32, tag=f"t3{c}")
                    nc.vector.tensor_add(t3[:], t2[:], Pni)
                    n = ep.tile([128, Bc], dt.float32, tag=f"n{c}")
                    nc.scalar.activation(n[:], t3[:], AF.Tanh, bias=bias[:, 2:3])
                    d = ep.tile([128, Bc], dt.float32, tag=f"d{c}")
                    nc.vector.tensor_sub(d[:], h_bf[c][:], n[:])
                    zd = ep.tile([128, Bc], dt.float32, tag=f"zd{c}")
                    nc.vector.tensor_mul(zd[:], z[:], d[:])
                    h_new = hp.tile([128, Bc], dt.bfloat16, tag=f"hs{c}")
                    nc.vector.tensor_add(h_new[:], n[:], zd[:])
                    h_bf[c] = h_new

                    cc_in = dp.tile([128, Bc], dt.bfloat16, tag=f"cin{c}")
                    nc.sync.dma_start(cc_in[:], h_new[:])
                    cc_out = dp.tile([NCORES * 128, Bc], dt.bfloat16, tag=f"cout{c}")
                    nc.gpsimd.collective_compute(
                        "AllGather",
                        mybir.AluOpType.bypass,
                        replica_groups=RG,
                        ins=[cc_in.opt()],
                        outs=[cc_out.opt()],
                    )
                    hk = []
                    for half in range(2):
                        ht = hp.tile([128, 4, Bc], dt.bfloat16, tag=f"hall{c}{half}")
                        nc.sync.dma_start(
                            ht[:],
                            cc_out[half * 512 : (half + 1) * 512, :].rearrange(
                                "(k p) n -> p k n", p=128
                            ),
                        )
                        hk.append(ht)
                    hall[c] = hk

                    Py = yp.tile([OSLICE, Bc], dt.float32, tag=f"py{c}")
                    for kt in range(K_REC):
                        nc.tensor.matmul(
                            Py[:],
                            wfc_sb[:, kt, :],
                            hk[kt // 4][:, kt % 4, :],
                            start=(kt == 0),
                            stop=(kt == K_REC - 1),
                        )
                    y_sb = ep.tile([OSLICE, Bc], dt.float32, tag=f"ysb{c}")
                    nc.scalar.activation(y_sb[:], Py[:], AF.Identity, bias=bfc_sb[:])
                    nc.sync.dma_start(out_d[t][:, col], y_sb[:])

    nc.compile()
    return nc


def _prep_inputs(src, hidden, W_ih, W_hh, b_ih, b_hh, W_fc, b_fc):
    from ml_dtypes import bfloat16

    f32 = np.float32
    src = np.asarray(src, f32)
    hidden = np.asarray(hidden, f32)
    W_ih = np.asarray(W_ih, f32)
    W_hh = np.asarray(W_hh, f32)
    b_ih = np.asarray(b_ih, f32)
    b_hh = np.asarray(b_hh, f32)
    W_fc = np.asarray(W_fc, f32)
    b_fc = np.asarray(b_fc, f32)

    x0 = src[0]  # [B, OUT]
    h0 = hidden[0]  # [B, H]

    W_comb = W_ih @ W_fc  # [3H, H]
    b_comb = b_ih + W_ih @ b_fc  # [3H]

    def to_ktiles(lhsT, m):  # [K, m] -> [128, K/128, m]
        k = lhsT.shape[0] // 128
        return np.ascontiguousarray(
            lhsT.reshape(k, 128, m).transpose(1, 0, 2)
        ).astype(bfloat16)

    in_maps = []
    for c in range(NCORES):
        Jk = slice(128 * c, 128 * c + 128)
        Zk = slice(H + 128 * c, H + 128 * c + 128)
        Nk = slice(2 * H + 128 * c, 2 * H + 128 * c + 128)
        Ok = slice(OSLICE * c, OSLICE * c + OSLICE)

        W_rec = np.concatenate(
            [
                W_comb[Jk] + W_hh[Jk],
                W_comb[Zk] + W_hh[Zk],
                W_comb[Nk],
                W_hh[Nk],
            ],
            axis=0,
        )  # [512, H]

        W0 = np.zeros((MSLICE, OUT + H), f32)
        W0[0:128, :OUT] = W_ih[Jk]
        W0[0:128, OUT:] = W_hh[Jk]
        W0[128:256, :OUT] = W_ih[Zk]
        W0[128:256, OUT:] = W_hh[Zk]
        W0[256:384, :OUT] = W_ih[Nk]
        W0[384:512, OUT:] = W_hh[Nk]

        rhs0 = np.concatenate([x0, h0], axis=1).T  # [OUT+H, B]

        biasS = np.stack(
            [
                b_comb[Jk] + b_hh[Jk],
                b_comb[Zk] + b_hh[Zk],
                b_comb[Nk],
                b_hh[Nk],
            ],
            axis=1,
        )  # [128, 4]
        bias0 = np.stack(
            [
                b_ih[Jk] + b_hh[Jk],
                b_ih[Zk] + b_hh[Zk],
                b_ih[Nk],
                b_hh[Nk],
            ],
            axis=1,
        )

        in_maps.append(
            {
                "w_rec": to_ktiles(W_rec.T, MSLICE),
                "w0": to_ktiles(W0.T, MSLICE),
                "wfc": to_ktiles(np.ascontiguousarray(W_fc[Ok]).T, OSLICE),
                "rhs0": to_ktiles(rhs0, B),
                "h0own": np.ascontiguousarray(h0[:, Jk].T).astype(bfloat16),
                "biasS": np.ascontiguousarray(biasS),
                "bias0": np.ascontiguousarray(bias0),
                "bfc": np.ascontiguousarray(b_fc[Ok].reshape(OSLICE, 1)),
            }
        )
    return in_maps


def kernel(src, tgt, hidden, W_ih, W_hh, b_ih, b_hh, W_fc, b_fc, **_unused):
    from concourse import bass_utils

    if "nc" not in _cache:
        _cache["nc"] = _build_program()
    nc = _cache["nc"]

    in_maps = _prep_inputs(src, hidden, W_ih, W_hh, b_ih, b_hh, W_fc, b_fc)
    res = bass_utils.run_bass_kernel_spmd(
        nc, in_maps, core_ids=list(range(NCORES))
    )
    # per-core out: [T, 96, B] -> full [T, B, OUT]
    outs = [np.asarray(r["out"]) for r in res.results]
    full = np.concatenate([o.transpose(0, 2, 1) for o in outs], axis=2)
    return np.ascontiguousarray(full.astype(np.float32))



# revision 2
# speedup vs baseline: 3.5175x; 3.5175x over previous
"""GRU decoder kernel for 8 trn2 NeuronCores — data-parallel, collective-free.

Derivation (same folding as before): x_{t+1} = y_t = h_{t+1} @ W_fc.T + b_fc,
so for t>=1 the input-path matmul folds into the recurrence:
  gi_t = h_t @ (W_ih @ W_fc).T + (b_ih + W_ih @ b_fc).
r/z gates use gi+gh, so those rows of the folded matrix and W_hh are summed
host-side; the n-gate keeps gi_n / gh_n separate (r multiplies only gh_n).

Step 0 (which needs x0 = src[0]) is computed exactly on the host in f32; the
device starts from h_1 and runs T-1 folded steps.

Sharding: pure data-parallel over batch (no collectives). Core c owns batch
columns [32c, 32c+32); weights are replicated. Each step is one
[4096, 1024] x [1024, 32] matmul (32 accumulation chains of 8 k-tiles) plus
elementwise. h_t^T stays in SBUF; each step's h is also DMA'd to a DRAM
scratch buffer, and the output projection y = h @ W_fc.T + b_fc runs as a
single batched matmul over all timesteps at the end. The time loop is a
hardware For_i loop (unrolled x2 for h double-buffering), so the program is
~1k instructions instead of ~80k — this slashes the per-call host cost
(tracing + NEFF re-compile) as well as on-device sequencing.
"""

import os
import sys

sys.path.insert(0, "/opt/trn_rl_repo")

import numpy as np

H = 1024
OUT = 768
B = 256
T = int(os.environ.get("GRU_T", "256"))
NCORES = 8
BL = B // NCORES  # 32 batch columns per core
KT = H // 128  # 8 k-tiles
G = 8  # hidden groups of 128 (= KT)
MT = OUT // 128  # 6 output m-tiles
T_DEV = T - 1  # steps computed on device
CHUNK = 2048  # fc columns per chunk (64 steps x 32 batch)
N_COLS = T_DEV * BL
N_CHUNKS = (N_COLS + CHUNK - 1) // CHUNK
PAD_COLS = N_CHUNKS * CHUNK

_cache = {}


def _build_program():
    import concourse.mybir as mybir
    from concourse import bacc, tile
    from concourse.bass import ts

    dt = mybir.dt
    AF = mybir.ActivationFunctionType
    OP = mybir.AluOpType

    nc = bacc.Bacc(num_devices=NCORES)

    w_d = nc.dram_tensor("w", [128, KT, 4 * H], dt.bfloat16, kind="ExternalInput")
    wfc_d = nc.dram_tensor("wfc", [128, KT, OUT], dt.bfloat16, kind="ExternalInput")
    h1_d = nc.dram_tensor("h1", [128, KT, BL], dt.bfloat16, kind="ExternalInput")
    bias_d = nc.dram_tensor("bias", [128, G, 4], dt.float32, kind="ExternalInput")
    bfc_d = nc.dram_tensor("bfc", [128, MT], dt.float32, kind="ExternalInput")
    out_d = nc.dram_tensor("out", [OUT, PAD_COLS], dt.bfloat16, kind="ExternalOutput")

    with tile.TileContext(nc) as tc:
        with (
            tc.tile_pool(name="wp", bufs=1) as wp,
            tc.tile_pool(name="hp", bufs=1) as hp,
            tc.tile_pool(name="ep", bufs=3) as ep,
            tc.tile_pool(name="pp", bufs=2, space="PSUM") as pp,
            tc.tile_pool(name="fp", bufs=2) as fp,
            tc.tile_pool(name="yp", bufs=2, space="PSUM") as yp,
            tc.tile_pool(name="dp", bufs=1, space="DRAM") as dp,
        ):
            w_sb = wp.tile([128, KT, 4 * H], dt.bfloat16)
            nc.sync.dma_start(w_sb[:], w_d[:])
            wfc_sb = wp.tile([128, KT, OUT], dt.bfloat16)
            nc.sync.dma_start(wfc_sb[:], wfc_d[:])
            bias_sb = wp.tile([128, G, 4], dt.float32)
            nc.sync.dma_start(bias_sb[:], bias_d[:])
            bfc_sb = wp.tile([128, MT], dt.float32)
            nc.sync.dma_start(bfc_sb[:], bfc_d[:])

            hA = hp.tile([128, KT, BL], dt.bfloat16, tag="hA")
            nc.sync.dma_start(hA[:], h1_d[:])
            hB = hp.tile([128, KT, BL], dt.bfloat16, tag="hB")

            H_d = dp.tile([128, KT, PAD_COLS], dt.bfloat16, tag="H")

            def step(src, dst, t_iv):
                # h_{t+1}^T = gru_folded(h_t^T); also DMA to H_d[:, :, t*BL:]
                P = pp.tile([128, 4 * G * BL], dt.float32, tag="P")
                for g in range(G):
                    for m in range(4):
                        col = (g * 4 + m) * BL
                        lhs_col = (g * 4 + m) * 128
                        for k in range(KT):
                            nc.tensor.matmul(
                                P[:, col : col + BL],
                                w_sb[:, k, lhs_col : lhs_col + 128],
                                src[:, k, :],
                                start=(k == 0),
                                stop=(k == KT - 1),
                            )
                    base = g * 4 * BL
                    Pr = P[:, base : base + BL]
                    Pz = P[:, base + BL : base + 2 * BL]
                    Pni = P[:, base + 2 * BL : base + 3 * BL]
                    Pnh = P[:, base + 3 * BL : base + 4 * BL]
                    r = ep.tile([128, BL], dt.float32, tag="r")
                    nc.scalar.activation(r[:], Pr, AF.Sigmoid, bias=bias_sb[:, g, 0:1])
                    z = ep.tile([128, BL], dt.float32, tag="z")
                    nc.scalar.activation(z[:], Pz, AF.Sigmoid, bias=bias_sb[:, g, 1:2])
                    t2 = ep.tile([128, BL], dt.float32, tag="t2")
                    nc.vector.scalar_tensor_tensor(
                        t2[:], Pnh, bias_sb[:, g, 3:4], r[:], OP.add, OP.mult
                    )
                    t3 = ep.tile([128, BL], dt.float32, tag="t3")
                    nc.vector.tensor_add(t3[:], t2[:], Pni)
                    n = ep.tile([128, BL], dt.float32, tag="n")
                    nc.scalar.activation(n[:], t3[:], AF.Tanh, bias=bias_sb[:, g, 2:3])
                    d = ep.tile([128, BL], dt.float32, tag="d")
                    nc.vector.tensor_sub(d[:], src[:, g, :], n[:])
                    zd = ep.tile([128, BL], dt.float32, tag="zd")
                    nc.vector.tensor_mul(zd[:], z[:], d[:])
                    nc.vector.tensor_add(dst[:, g, :], n[:], zd[:])
                nc.sync.dma_start(H_d[:, :, ts(t_iv, BL)], dst[:])

            n_pairs = T_DEV // 2
            if n_pairs > 0:
                with tc.For_i(0, 2 * n_pairs, 2) as iv:
                    step(hA, hB, iv)
                    step(hB, hA, iv + 1)
            for t in range(2 * n_pairs, T_DEV):
                step(hA, hB, t)

            # Batched output projection over all timesteps:
            # Y^T[768, t*BL+n] = W_fc @ H^T (+ b_fc), in chunks of CHUNK cols.
            with tc.For_i(0, N_CHUNKS) as civ:
                Hc = fp.tile([128, KT, CHUNK], dt.bfloat16, tag="Hc")
                nc.sync.dma_start(Hc[:], H_d[:, :, ts(civ, CHUNK)])
                for m in range(MT):
                    for nb in range(CHUNK // 512):
                        Py = yp.tile([128, 512], dt.float32, tag="Py")
                        for k in range(KT):
                            nc.tensor.matmul(
                                Py[:],
                                wfc_sb[:, k, m * 128 : (m + 1) * 128],
                                Hc[:, k, nb * 512 : (nb + 1) * 512],
                                start=(k == 0),
                                stop=(k == KT - 1),
                            )
                        y_sb = fp.tile([128, 512], dt.bfloat16, tag="y")
                        nc.scalar.activation(
                            y_sb[:], Py[:], AF.Identity, bias=bfc_sb[:, m : m + 1]
                        )
                        nc.sync.dma_start(
                            out_d[
                                m * 128 : (m + 1) * 128,
                                ts(civ * (CHUNK // 512) + nb, 512),
                            ],
                            y_sb[:],
                        )

    nc.compile()
    return nc


def _sigmoid(v):
    return 1.0 / (1.0 + np.exp(-v))


def _to_ktiles(lhsT, m):  # [K, m] -> [128, K/128, m]
    k = lhsT.shape[0] // 128
    return np.ascontiguousarray(lhsT.reshape(k, 128, m).transpose(1, 0, 2))


def _prep_inputs(src, hidden, W_ih, W_hh, b_ih, b_hh, W_fc, b_fc):
    from ml_dtypes import bfloat16

    f32 = np.float32
    src = np.asarray(src, f32)
    hidden = np.asarray(hidden, f32)
    W_ih = np.asarray(W_ih, f32)
    W_hh = np.asarray(W_hh, f32)
    b_ih = np.asarray(b_ih, f32)
    b_hh = np.asarray(b_hh, f32)
    W_fc = np.asarray(W_fc, f32)
    b_fc = np.asarray(b_fc, f32)

    x0 = src[0]  # [B, OUT]
    h0 = hidden[0]  # [B, H]

    # exact f32 step 0 on host
    gi = x0 @ W_ih.T + b_ih
    gh = h0 @ W_hh.T + b_hh
    r0 = _sigmoid(gi[:, :H] + gh[:, :H])
    z0 = _sigmoid(gi[:, H : 2 * H] + gh[:, H : 2 * H])
    n0 = np.tanh(gi[:, 2 * H :] + r0 * gh[:, 2 * H :])
    h1 = (1.0 - z0) * n0 + z0 * h0  # [B, H]
    y0 = h1 @ W_fc.T + b_fc  # [B, OUT]

    W_comb = W_ih @ W_fc  # [3H, H]
    b_comb = b_ih + W_ih @ b_fc

    Wr = W_comb[:H] + W_hh[:H]
    Wz = W_comb[H : 2 * H] + W_hh[H : 2 * H]
    Wni = W_comb[2 * H :]
    Wnh = W_hh[2 * H :]
    Wbig = np.empty((4 * H, H), f32)
    for g in range(G):
        base = g * 512
        Wbig[base : base + 128] = Wr[g * 128 : (g + 1) * 128]
        Wbig[base + 128 : base + 256] = Wz[g * 128 : (g + 1) * 128]
        Wbig[base + 256 : base + 384] = Wni[g * 128 : (g + 1) * 128]
        Wbig[base + 384 : base + 512] = Wnh[g * 128 : (g + 1) * 128]
    w = _to_ktiles(Wbig.T, 4 * H).astype(bfloat16)
    wfc = _to_ktiles(np.ascontiguousarray(W_fc).T, OUT).astype(bfloat16)

    br = b_comb[:H] + b_hh[:H]
    bz = b_comb[H : 2 * H] + b_hh[H : 2 * H]
    bni = b_comb[2 * H :]
    bnh = b_hh[2 * H :]
    bias = np.stack(
        [b.reshape(G, 128) for b in (br, bz, bni, bnh)], axis=-1
    ).transpose(1, 0, 2)  # [128, G, 4]
    bias = np.ascontiguousarray(bias.astype(f32))
    bfc = np.ascontiguousarray(b_fc.reshape(MT, 128).T.astype(f32))

    in_maps = []
    for c in range(NCORES):
        h1c = _to_ktiles(
            np.ascontiguousarray(h1[c * BL : (c + 1) * BL].T), BL
        ).astype(bfloat16)
        in_maps.append(
            {"w": w, "wfc": wfc, "h1": h1c, "bias": bias, "bfc": bfc}
        )
    return in_maps, y0


def kernel(src, tgt, hidden, W_ih, W_hh, b_ih, b_hh, W_fc, b_fc, **_unused):
    from concourse import bass_utils

    if "nc" not in _cache:
        _cache["nc"] = _build_program()
    nc = _cache["nc"]

    in_maps, y0 = _prep_inputs(src, hidden, W_ih, W_hh, b_ih, b_hh, W_fc, b_fc)
    res = bass_utils.run_bass_kernel_spmd(nc, in_maps, core_ids=list(range(NCORES)))
    full = np.empty((T, B, OUT), np.float32)
    full[0] = y0
    for c in range(NCORES):
        yc = np.asarray(res.results[c]["out"])[:, :N_COLS].astype(np.float32)
        full[1:, c * BL : (c + 1) * BL, :] = yc.reshape(OUT, T_DEV, BL).transpose(
            1, 2, 0
        )
    return full


# revision 6
# speedup vs baseline: 4.2954x; 1.2212x over previous
"""GRU decoder kernel for 8 trn2 NeuronCores — data-parallel, transfer-optimized.

Derivation: x_{t+1} = y_t = h_{t+1} @ W_fc.T + b_fc, so for t>=1 the
input-path matmul folds into the recurrence:
  gi_t = h_t @ (W_ih @ W_fc).T + (b_ih + W_ih @ b_fc).
r/z gates use gi+gh, so those rows of the folded matrix and W_hh are summed
host-side; the n-gate keeps gi_n / gh_n separate (r multiplies only gh_n).
Step 0 (needs x0 = src[0]) is computed exactly on the host in f32; the device
starts from h_1 and runs T-1 folded steps.

Sharding: data-parallel over batch (no per-step collectives). Core c owns
batch columns [32c, 32c+32). The time loop is a hardware For_i loop
(program ~1k instructions). h_t^T is DMA'd per step to a DRAM scratch; the
output projection runs as one batched matmul at the end.

The wall clock is dominated by the axon tunnel (~90 MB/s up, ~50 MB/s down),
so this version minimizes wire bytes:
  - weights are uploaded SHARDED (each core gets 1/8 of the folded W and of
    W_fc) and rebuilt on-device with one startup AllGather: 76 MB -> 10 MB.
  - the output is int8 with per-(row, 512-col-block) scales computed on
    device (absmax over each block row); the host dequantizes. This halves
    both the output download and the donated zero-buffer upload.
"""

import os
import sys

sys.path.insert(0, "/opt/trn_rl_repo")

import numpy as np

H = 1024
OUT = 768
B = 256
T = int(os.environ.get("GRU_T", "256"))
NCORES = 8
BL = B // NCORES  # 32 batch columns per core
KT = H // 128  # 8 k-tiles
G = 8  # hidden groups of 128 (= KT)
MT = OUT // 128  # 6 output m-tiles
T_DEV = T - 1  # steps computed on device
UNROLL = int(os.environ.get("GRU_UNROLL", "2"))  # steps per For_i iteration (even)
CHUNK = 2048  # fc columns per chunk (64 steps x 32 batch)
NB = CHUNK // 512
N_COLS = T_DEV * BL
N_CHUNKS = (N_COLS + CHUNK - 1) // CHUNK
PAD_COLS = N_CHUNKS * CHUNK
MSH = 4 * H // NCORES  # 512 folded-weight cols per core shard
OSH = OUT // NCORES  # 96 fc cols per core shard
QMAX = 126.5  # int8 headroom so the max element never wraps

_cache = {}


def _build_program():
    import concourse.mybir as mybir
    from concourse import bacc, tile
    from concourse.bass import ds, ts

    dt = mybir.dt
    AF = mybir.ActivationFunctionType
    OP = mybir.AluOpType
    RG = [list(range(NCORES))]

    nc = bacc.Bacc(num_devices=NCORES)

    wsh_d = nc.dram_tensor("wsh", [128, KT, MSH], dt.bfloat16, kind="ExternalInput")
    wfsh_d = nc.dram_tensor("wfsh", [128, KT, OSH], dt.bfloat16, kind="ExternalInput")
    h1_d = nc.dram_tensor("h1", [128, KT, BL], dt.bfloat16, kind="ExternalInput")
    bias_d = nc.dram_tensor("bias", [128, G, 4], dt.float32, kind="ExternalInput")
    bfc_d = nc.dram_tensor("bfc", [128, MT], dt.float32, kind="ExternalInput")
    out_d = nc.dram_tensor("out", [OUT, PAD_COLS], dt.int8, kind="ExternalOutput")
    s_d = nc.dram_tensor(
        "s", [OUT, N_CHUNKS * NB], dt.float32, kind="ExternalOutput"
    )

    with tile.TileContext(nc) as tc:
        with (
            tc.tile_pool(name="wp", bufs=1) as wp,
            tc.tile_pool(name="hp", bufs=1) as hp,
            tc.tile_pool(name="ep", bufs=3) as ep,
            tc.tile_pool(name="pp", bufs=2, space="PSUM") as pp,
            tc.tile_pool(name="fp", bufs=2) as fp,
            tc.tile_pool(name="yp", bufs=2, space="PSUM") as yp,
            tc.tile_pool(name="dp", bufs=1, space="DRAM") as dp,
        ):
            # --- rebuild replicated weights from shards with one AllGather ---
            w_sb = wp.tile([128, KT, 4 * H], dt.bfloat16)
            wfc_sb = wp.tile([128, KT, OUT], dt.bfloat16)

            wst = fp.tile([128, KT, MSH], dt.bfloat16, tag="wst")
            nc.sync.dma_start(wst[:], wsh_d[:])
            wfst = fp.tile([128, KT, OSH], dt.bfloat16, tag="wfst")
            nc.sync.dma_start(wfst[:], wfsh_d[:])

            cc_in = dp.tile([128, KT, MSH], dt.bfloat16, tag="ccin")
            nc.sync.dma_start(cc_in[:], wst[:])
            cc_out = dp.tile([NCORES * 128, KT, MSH], dt.bfloat16, tag="ccout")
            nc.gpsimd.collective_compute(
                "AllGather",
                mybir.AluOpType.bypass,
                replica_groups=RG,
                ins=[cc_in.opt()],
                outs=[cc_out.opt()],
            )
            cc_in2 = dp.tile([128, KT, OSH], dt.bfloat16, tag="ccin2")
            nc.sync.dma_start(cc_in2[:], wfst[:])
            cc_out2 = dp.tile([NCORES * 128, KT, OSH], dt.bfloat16, tag="ccout2")
            nc.gpsimd.collective_compute(
                "AllGather",
                mybir.AluOpType.bypass,
                replica_groups=RG,
                ins=[cc_in2.opt()],
                outs=[cc_out2.opt()],
            )
            for c in range(NCORES):
                nc.sync.dma_start(
                    w_sb[:, :, c * MSH : (c + 1) * MSH],
                    cc_out[c * 128 : (c + 1) * 128],
                )
                nc.sync.dma_start(
                    wfc_sb[:, :, c * OSH : (c + 1) * OSH],
                    cc_out2[c * 128 : (c + 1) * 128],
                )

            bias_sb = wp.tile([128, G, 4], dt.float32)
            nc.sync.dma_start(bias_sb[:], bias_d[:])
            bfc_sb = wp.tile([128, MT], dt.float32)
            nc.sync.dma_start(bfc_sb[:], bfc_d[:])

            hA = hp.tile([128, KT, BL], dt.bfloat16, tag="hA")
            nc.sync.dma_start(hA[:], h1_d[:])
            hB = hp.tile([128, KT, BL], dt.bfloat16, tag="hB")

            H_d = dp.tile([128, KT, PAD_COLS], dt.bfloat16, tag="H")
            # zero H_d's padding columns so fc-block absmax sees no garbage
            if PAD_COLS > N_COLS:
                zt = fp.tile([128, KT, min(512, PAD_COLS - N_COLS)], dt.bfloat16,
                             tag="zt")
                nc.vector.memset(zt[:], 0.0)
                zoff = N_COLS
                while zoff < PAD_COLS:
                    zw = min(512, PAD_COLS - zoff)
                    nc.sync.dma_start(H_d[:, :, zoff : zoff + zw], zt[:, :, :zw])
                    zoff += zw

            def step(src, dst, t_iv):
                # h_{t+1}^T = gru_folded(h_t^T); also DMA to H_d[:, :, t*BL:]
                P = pp.tile([128, 4 * G * BL], dt.float32, tag="P")
                for g in range(G):
                    for m in range(4):
                        col = (g * 4 + m) * BL
                        lhs_col = (g * 4 + m) * 128
                        for k in range(KT):
                            nc.tensor.matmul(
                                P[:, col : col + BL],
                                w_sb[:, k, lhs_col : lhs_col + 128],
                                src[:, k, :],
                                start=(k == 0),
                                stop=(k == KT - 1),
                            )
                    base = g * 4 * BL
                    Pr = P[:, base : base + BL]
                    Pz = P[:, base + BL : base + 2 * BL]
                    Pni = P[:, base + 2 * BL : base + 3 * BL]
                    Pnh = P[:, base + 3 * BL : base + 4 * BL]
                    r = ep.tile([128, BL], dt.float32, tag="r")
                    nc.scalar.activation(r[:], Pr, AF.Sigmoid, bias=bias_sb[:, g, 0:1])
                    z = ep.tile([128, BL], dt.float32, tag="z")
                    nc.scalar.activation(z[:], Pz, AF.Sigmoid, bias=bias_sb[:, g, 1:2])
                    t2 = ep.tile([128, BL], dt.float32, tag="t2")
                    nc.vector.scalar_tensor_tensor(
                        t2[:], Pnh, bias_sb[:, g, 3:4], r[:], OP.add, OP.mult
                    )
                    t3 = ep.tile([128, BL], dt.float32, tag="t3")
                    nc.vector.tensor_add(t3[:], t2[:], Pni)
                    n = ep.tile([128, BL], dt.float32, tag="n")
                    nc.scalar.activation(n[:], t3[:], AF.Tanh, bias=bias_sb[:, g, 2:3])
                    d = ep.tile([128, BL], dt.float32, tag="d")
                    nc.vector.tensor_sub(d[:], src[:, g, :], n[:])
                    zd = ep.tile([128, BL], dt.float32, tag="zd")
                    nc.vector.tensor_mul(zd[:], z[:], d[:])
                    nc.vector.tensor_add(dst[:, g, :], n[:], zd[:])
                nc.sync.dma_start(H_d[:, :, ts(t_iv, BL)], dst[:])

            n_iters = T_DEV // UNROLL
            if n_iters > 0:
                with tc.For_i(0, n_iters * UNROLL, UNROLL) as iv:
                    for u in range(UNROLL):
                        sd = (hA, hB) if u % 2 == 0 else (hB, hA)
                        step(sd[0], sd[1], iv + u)
            for t in range(n_iters * UNROLL, T_DEV):
                sd = (hA, hB) if t % 2 == 0 else (hB, hA)
                step(sd[0], sd[1], t)

            # Batched output projection, quantized to int8 with per-row
            # per-512-col-block scales (absmax-based, computed on device).
            with tc.For_i(0, N_CHUNKS) as civ:
                Hc = fp.tile([128, KT, CHUNK], dt.bfloat16, tag="Hc")
                nc.sync.dma_start(Hc[:], H_d[:, :, ts(civ, CHUNK)])
                for m in range(MT):
                    for nb in range(NB):
                        Py = yp.tile([128, 512], dt.float32, tag="Py")
                        for k in range(KT):
                            nc.tensor.matmul(
                                Py[:],
                                wfc_sb[:, k, m * 128 : (m + 1) * 128],
                                Hc[:, k, nb * 512 : (nb + 1) * 512],
                                start=(k == 0),
                                stop=(k == KT - 1),
                            )
                        yf = fp.tile([128, 512], dt.float32, tag="yf")
                        nc.scalar.activation(
                            yf[:], Py[:], AF.Identity, bias=bfc_sb[:, m : m + 1]
                        )
                        sraw = ep.tile([128, 1], dt.float32, tag="sraw")
                        nc.vector.tensor_reduce(
                            sraw[:], yf[:], mybir.AxisListType.X, OP.max,
                            apply_absolute_value=True,
                        )
                        sq = ep.tile([128, 1], dt.float32, tag="sq")
                        nc.vector.tensor_scalar(
                            sq[:], sraw[:], 1.0 / QMAX, 1e-30, OP.mult, OP.add
                        )
                        inv = ep.tile([128, 1], dt.float32, tag="inv")
                        nc.vector.reciprocal(inv[:], sq[:])
                        yq = fp.tile([128, 512], dt.int8, tag="yq")
                        nc.scalar.activation(
                            yq[:], yf[:], AF.Identity, scale=inv[:, 0:1]
                        )
                        nc.sync.dma_start(
                            out_d[m * 128 : (m + 1) * 128, ts(civ * NB + nb, 512)],
                            yq[:],
                        )
                        nc.sync.dma_start(
                            s_d[m * 128 : (m + 1) * 128, ds(civ * NB + nb, 1)],
                            sq[:],
                        )

    nc.compile()
    return nc


def _sigmoid(v):
    return 1.0 / (1.0 + np.exp(-v))


def _to_ktiles(lhsT, m):  # [K, m] -> [128, K/128, m]
    k = lhsT.shape[0] // 128
    return np.ascontiguousarray(lhsT.reshape(k, 128, m).transpose(1, 0, 2))


def _prep_inputs(src, hidden, W_ih, W_hh, b_ih, b_hh, W_fc, b_fc):
    from ml_dtypes import bfloat16

    f32 = np.float32
    src = np.asarray(src, f32)
    hidden = np.asarray(hidden, f32)
    W_ih = np.asarray(W_ih, f32)
    W_hh = np.asarray(W_hh, f32)
    b_ih = np.asarray(b_ih, f32)
    b_hh = np.asarray(b_hh, f32)
    W_fc = np.asarray(W_fc, f32)
    b_fc = np.asarray(b_fc, f32)

    x0 = src[0]  # [B, OUT]
    h0 = hidden[0]  # [B, H]

    # exact f32 step 0 on host
    gi = x0 @ W_ih.T + b_ih
    gh = h0 @ W_hh.T + b_hh
    r0 = _sigmoid(gi[:, :H] + gh[:, :H])
    z0 = _sigmoid(gi[:, H : 2 * H] + gh[:, H : 2 * H])
    n0 = np.tanh(gi[:, 2 * H :] + r0 * gh[:, 2 * H :])
    h1 = (1.0 - z0) * n0 + z0 * h0  # [B, H]
    y0 = h1 @ W_fc.T + b_fc  # [B, OUT]

    W_comb = W_ih @ W_fc  # [3H, H]
    b_comb = b_ih + W_ih @ b_fc

    Wr = W_comb[:H] + W_hh[:H]
    Wz = W_comb[H : 2 * H] + W_hh[H : 2 * H]
    Wni = W_comb[2 * H :]
    Wnh = W_hh[2 * H :]
    Wbig = np.empty((4 * H, H), f32)
    for g in range(G):
        base = g * 512
        Wbig[base : base + 128] = Wr[g * 128 : (g + 1) * 128]
        Wbig[base + 128 : base + 256] = Wz[g * 128 : (g + 1) * 128]
        Wbig[base + 256 : base + 384] = Wni[g * 128 : (g + 1) * 128]
        Wbig[base + 384 : base + 512] = Wnh[g * 128 : (g + 1) * 128]
    w = _to_ktiles(Wbig.T, 4 * H).astype(bfloat16)  # [128, KT, 4H]
    wfc = _to_ktiles(np.ascontiguousarray(W_fc).T, OUT).astype(bfloat16)

    br = b_comb[:H] + b_hh[:H]
    bz = b_comb[H : 2 * H] + b_hh[H : 2 * H]
    bni = b_comb[2 * H :]
    bnh = b_hh[2 * H :]
    bias = np.stack(
        [b.reshape(G, 128) for b in (br, bz, bni, bnh)], axis=-1
    ).transpose(1, 0, 2)  # [128, G, 4]
    bias = np.ascontiguousarray(bias.astype(f32))
    bfc = np.ascontiguousarray(b_fc.reshape(MT, 128).T.astype(f32))

    in_maps = []
    for c in range(NCORES):
        h1c = _to_ktiles(
            np.ascontiguousarray(h1[c * BL : (c + 1) * BL].T), BL
        ).astype(bfloat16)
        in_maps.append(
            {
                "wsh": np.ascontiguousarray(w[:, :, c * MSH : (c + 1) * MSH]),
                "wfsh": np.ascontiguousarray(wfc[:, :, c * OSH : (c + 1) * OSH]),
                "h1": h1c,
                "bias": bias,
                "bfc": bfc,
            }
        )
    return in_maps, y0


def kernel(src, tgt, hidden, W_ih, W_hh, b_ih, b_hh, W_fc, b_fc, **_unused):
    from concourse import bass_utils

    if "nc" not in _cache:
        _cache["nc"] = _build_program()
    nc = _cache["nc"]

    in_maps, y0 = _prep_inputs(src, hidden, W_ih, W_hh, b_ih, b_hh, W_fc, b_fc)
    res = bass_utils.run_bass_kernel_spmd(nc, in_maps, core_ids=list(range(NCORES)))
    full = np.empty((T, B, OUT), np.float32)
    full[0] = y0
    for c in range(NCORES):
        yq = np.asarray(res.results[c]["out"]).astype(np.float32)  # [OUT, PAD]
        s = np.asarray(res.results[c]["s"])  # [OUT, N_CHUNKS*NB]
        yc = (yq.reshape(OUT, N_CHUNKS * NB, 512) * s[:, :, None]).reshape(
            OUT, PAD_COLS
        )[:, :N_COLS]
        full[1:, c * BL : (c + 1) * BL, :] = yc.reshape(OUT, T_DEV, BL).transpose(
            1, 2, 0
        )
    return full


# revision 7
# speedup vs baseline: 4.7916x; 1.1155x over previous
"""GRU decoder kernel for 8 trn2 NeuronCores — data-parallel, transfer-optimized.

Derivation: x_{t+1} = y_t = h_{t+1} @ W_fc.T + b_fc, so for t>=1 the
input-path matmul folds into the recurrence:
  gi_t = h_t @ (W_ih @ W_fc).T + (b_ih + W_ih @ b_fc).
r/z gates use gi+gh, so those rows of the folded matrix and W_hh are summed
host-side; the n-gate keeps gi_n / gh_n separate (r multiplies only gh_n).
Step 0 (needs x0 = src[0]) is computed exactly on the host in f32; the device
starts from h_1 and runs T-1 folded steps.

Sharding: data-parallel over batch (no per-step collectives). Core c owns
batch columns [32c, 32c+32). The time loop is a hardware For_i loop
(program ~1k instructions). h_t^T is DMA'd per step to a DRAM scratch; the
output projection runs as one batched matmul at the end.

The wall clock is dominated by the axon tunnel (~90 MB/s up, ~50 MB/s down),
so this version minimizes wire bytes:
  - weights are uploaded SHARDED (each core gets 1/8 of the folded W and of
    W_fc) and rebuilt on-device with one startup AllGather: 76 MB -> 10 MB.
  - the output is int8 with per-(row, 512-col-block) scales computed on
    device (absmax over each block row); the host dequantizes. This halves
    both the output download and the donated zero-buffer upload.
"""

import os
import sys

sys.path.insert(0, "/opt/trn_rl_repo")

import numpy as np

try:  # persistent XLA compile cache: skips ~0.2s/call of re-compile
    import jax

    jax.config.update("jax_compilation_cache_dir", "/tmp/jax_comp_cache")
    jax.config.update("jax_persistent_cache_min_entry_size_bytes", -1)
    jax.config.update("jax_persistent_cache_min_compile_time_secs", 0)
except Exception:
    pass

H = 1024
OUT = 768
B = 256
T = int(os.environ.get("GRU_T", "256"))
NCORES = 8
BL = B // NCORES  # 32 batch columns per core
KT = H // 128  # 8 k-tiles
G = 8  # hidden groups of 128 (= KT)
MT = OUT // 128  # 6 output m-tiles
T_DEV = T - 1  # steps computed on device
UNROLL = int(os.environ.get("GRU_UNROLL", "2"))  # steps per For_i iteration (even)
CHUNK = 2048  # fc columns per chunk (64 steps x 32 batch)
NB = CHUNK // 512
N_COLS = T_DEV * BL
N_CHUNKS = (N_COLS + CHUNK - 1) // CHUNK
PAD_COLS = N_CHUNKS * CHUNK
MSH = 4 * H // NCORES  # 512 folded-weight cols per core shard
OSH = OUT // NCORES  # 96 fc cols per core shard
QMAX = 126.5  # int8 headroom so the max element never wraps

_cache = {}


def _build_program():
    import concourse.mybir as mybir
    from concourse import bacc, tile
    from concourse.bass import ds, ts

    dt = mybir.dt
    AF = mybir.ActivationFunctionType
    OP = mybir.AluOpType
    RG = [list(range(NCORES))]

    nc = bacc.Bacc(num_devices=NCORES)

    wsh_d = nc.dram_tensor("wsh", [128, KT, MSH], dt.bfloat16, kind="ExternalInput")
    wfsh_d = nc.dram_tensor("wfsh", [128, KT, OSH], dt.bfloat16, kind="ExternalInput")
    h1_d = nc.dram_tensor("h1", [128, KT, BL], dt.bfloat16, kind="ExternalInput")
    bias_d = nc.dram_tensor("bias", [128, G, 4], dt.float32, kind="ExternalInput")
    bfc_d = nc.dram_tensor("bfc", [128, MT], dt.float32, kind="ExternalInput")
    out_d = nc.dram_tensor("out", [OUT, PAD_COLS], dt.int8, kind="ExternalOutput")
    s_d = nc.dram_tensor(
        "s", [OUT, N_CHUNKS * NB], dt.float32, kind="ExternalOutput"
    )

    with tile.TileContext(nc) as tc:
        with (
            tc.tile_pool(name="wp", bufs=1) as wp,
            tc.tile_pool(name="hp", bufs=1) as hp,
            tc.tile_pool(name="ep", bufs=3) as ep,
            tc.tile_pool(name="pp", bufs=2, space="PSUM") as pp,
            tc.tile_pool(name="fp", bufs=2) as fp,
            tc.tile_pool(name="yp", bufs=2, space="PSUM") as yp,
            tc.tile_pool(name="dp", bufs=1, space="DRAM") as dp,
        ):
            # --- rebuild replicated weights from shards with one AllGather ---
            w_sb = wp.tile([128, KT, 4 * H], dt.bfloat16)
            wfc_sb = wp.tile([128, KT, OUT], dt.bfloat16)

            wst = fp.tile([128, KT, MSH], dt.bfloat16, tag="wst")
            nc.sync.dma_start(wst[:], wsh_d[:])
            wfst = fp.tile([128, KT, OSH], dt.bfloat16, tag="wfst")
            nc.sync.dma_start(wfst[:], wfsh_d[:])

            cc_in = dp.tile([128, KT, MSH], dt.bfloat16, tag="ccin")
            nc.sync.dma_start(cc_in[:], wst[:])
            cc_out = dp.tile([NCORES * 128, KT, MSH], dt.bfloat16, tag="ccout")
            nc.gpsimd.collective_compute(
                "AllGather",
                mybir.AluOpType.bypass,
                replica_groups=RG,
                ins=[cc_in.opt()],
                outs=[cc_out.opt()],
            )
            cc_in2 = dp.tile([128, KT, OSH], dt.bfloat16, tag="ccin2")
            nc.sync.dma_start(cc_in2[:], wfst[:])
            cc_out2 = dp.tile([NCORES * 128, KT, OSH], dt.bfloat16, tag="ccout2")
            nc.gpsimd.collective_compute(
                "AllGather",
                mybir.AluOpType.bypass,
                replica_groups=RG,
                ins=[cc_in2.opt()],
                outs=[cc_out2.opt()],
            )
            for c in range(NCORES):
                nc.sync.dma_start(
                    w_sb[:, :, c * MSH : (c + 1) * MSH],
                    cc_out[c * 128 : (c + 1) * 128],
                )
                nc.sync.dma_start(
                    wfc_sb[:, :, c * OSH : (c + 1) * OSH],
                    cc_out2[c * 128 : (c + 1) * 128],
                )

            bias_sb = wp.tile([128, G, 4], dt.float32)
            nc.sync.dma_start(bias_sb[:], bias_d[:])
            bfc_sb = wp.tile([128, MT], dt.float32)
            nc.sync.dma_start(bfc_sb[:], bfc_d[:])

            hA = hp.tile([128, KT, BL], dt.bfloat16, tag="hA")
            nc.sync.dma_start(hA[:], h1_d[:])
            hB = hp.tile([128, KT, BL], dt.bfloat16, tag="hB")

            H_d = dp.tile([128, KT, PAD_COLS], dt.bfloat16, tag="H")
            # zero H_d's padding columns so fc-block absmax sees no garbage
            if PAD_COLS > N_COLS:
                zt = fp.tile([128, KT, min(512, PAD_COLS - N_COLS)], dt.bfloat16,
                             tag="zt")
                nc.vector.memset(zt[:], 0.0)
                zoff = N_COLS
                while zoff < PAD_COLS:
                    zw = min(512, PAD_COLS - zoff)
                    nc.sync.dma_start(H_d[:, :, zoff : zoff + zw], zt[:, :, :zw])
                    zoff += zw

            def step(src, dst, t_iv):
                # h_{t+1}^T = gru_folded(h_t^T); also DMA to H_d[:, :, t*BL:]
                P = pp.tile([128, 4 * G * BL], dt.float32, tag="P")
                for g in range(G):
                    for m in range(4):
                        col = (g * 4 + m) * BL
                        lhs_col = (g * 4 + m) * 128
                        for k in range(KT):
                            nc.tensor.matmul(
                                P[:, col : col + BL],
                                w_sb[:, k, lhs_col : lhs_col + 128],
                                src[:, k, :],
                                start=(k == 0),
                                stop=(k == KT - 1),
                            )
                    base = g * 4 * BL
                    Pr = P[:, base : base + BL]
                    Pz = P[:, base + BL : base + 2 * BL]
                    Pni = P[:, base + 2 * BL : base + 3 * BL]
                    Pnh = P[:, base + 3 * BL : base + 4 * BL]
                    r = ep.tile([128, BL], dt.float32, tag="r")
                    nc.scalar.activation(r[:], Pr, AF.Sigmoid, bias=bias_sb[:, g, 0:1])
                    z = ep.tile([128, BL], dt.float32, tag="z")
                    nc.scalar.activation(z[:], Pz, AF.Sigmoid, bias=bias_sb[:, g, 1:2])
                    t2 = ep.tile([128, BL], dt.float32, tag="t2")
                    nc.vector.scalar_tensor_tensor(
                        t2[:], Pnh, bias_sb[:, g, 3:4], r[:], OP.add, OP.mult
                    )
                    t3 = ep.tile([128, BL], dt.float32, tag="t3")
                    nc.vector.tensor_add(t3[:], t2[:], Pni)
                    n = ep.tile([128, BL], dt.float32, tag="n")
                    nc.scalar.activation(n[:], t3[:], AF.Tanh, bias=bias_sb[:, g, 2:3])
                    d = ep.tile([128, BL], dt.float32, tag="d")
                    nc.vector.tensor_sub(d[:], src[:, g, :], n[:])
                    zd = ep.tile([128, BL], dt.float32, tag="zd")
                    nc.vector.tensor_mul(zd[:], z[:], d[:])
                    nc.vector.tensor_add(dst[:, g, :], n[:], zd[:])
                nc.sync.dma_start(H_d[:, :, ts(t_iv, BL)], dst[:])

            n_iters = T_DEV // UNROLL
            if n_iters > 0:
                with tc.For_i(0, n_iters * UNROLL, UNROLL) as iv:
                    for u in range(UNROLL):
                        sd = (hA, hB) if u % 2 == 0 else (hB, hA)
                        step(sd[0], sd[1], iv + u)
            for t in range(n_iters * UNROLL, T_DEV):
                sd = (hA, hB) if t % 2 == 0 else (hB, hA)
                step(sd[0], sd[1], t)

            # Batched output projection, quantized to int8 with per-row
            # per-512-col-block scales (absmax-based, computed on device).
            with tc.For_i(0, N_CHUNKS) as civ:
                Hc = fp.tile([128, KT, CHUNK], dt.bfloat16, tag="Hc")
                nc.sync.dma_start(Hc[:], H_d[:, :, ts(civ, CHUNK)])
                for m in range(MT):
                    for nb in range(NB):
                        Py = yp.tile([128, 512], dt.float32, tag="Py")
                        for k in range(KT):
                            nc.tensor.matmul(
                                Py[:],
                                wfc_sb[:, k, m * 128 : (m + 1) * 128],
                                Hc[:, k, nb * 512 : (nb + 1) * 512],
                                start=(k == 0),
                                stop=(k == KT - 1),
                            )
                        yf = fp.tile([128, 512], dt.float32, tag="yf")
                        nc.scalar.activation(
                            yf[:], Py[:], AF.Identity, bias=bfc_sb[:, m : m + 1]
                        )
                        sraw = ep.tile([128, 1], dt.float32, tag="sraw")
                        nc.vector.tensor_reduce(
                            sraw[:], yf[:], mybir.AxisListType.X, OP.max,
                            apply_absolute_value=True,
                        )
                        sq = ep.tile([128, 1], dt.float32, tag="sq")
                        nc.vector.tensor_scalar(
                            sq[:], sraw[:], 1.0 / QMAX, 1e-30, OP.mult, OP.add
                        )
                        inv = ep.tile([128, 1], dt.float32, tag="inv")
                        nc.vector.reciprocal(inv[:], sq[:])
                        yq = fp.tile([128, 512], dt.int8, tag="yq")
                        nc.scalar.activation(
                            yq[:], yf[:], AF.Identity, scale=inv[:, 0:1]
                        )
                        nc.sync.dma_start(
                            out_d[m * 128 : (m + 1) * 128, ts(civ * NB + nb, 512)],
                            yq[:],
                        )
                        nc.sync.dma_start(
                            s_d[m * 128 : (m + 1) * 128, ds(civ * NB + nb, 1)],
                            sq[:],
                        )

    nc.compile()
    return nc


def _sigmoid(v):
    return 1.0 / (1.0 + np.exp(-v))


def _to_ktiles(lhsT, m):  # [K, m] -> [128, K/128, m]
    k = lhsT.shape[0] // 128
    return np.ascontiguousarray(lhsT.reshape(k, 128, m).transpose(1, 0, 2))


def _prep_inputs(src, hidden, W_ih, W_hh, b_ih, b_hh, W_fc, b_fc):
    from ml_dtypes import bfloat16

    f32 = np.float32
    src = np.asarray(src, f32)
    hidden = np.asarray(hidden, f32)
    W_ih = np.asarray(W_ih, f32)
    W_hh = np.asarray(W_hh, f32)
    b_ih = np.asarray(b_ih, f32)
    b_hh = np.asarray(b_hh, f32)
    W_fc = np.asarray(W_fc, f32)
    b_fc = np.asarray(b_fc, f32)

    x0 = src[0]  # [B, OUT]
    h0 = hidden[0]  # [B, H]

    # exact f32 step 0 on host
    gi = x0 @ W_ih.T + b_ih
    gh = h0 @ W_hh.T + b_hh
    r0 = _sigmoid(gi[:, :H] + gh[:, :H])
    z0 = _sigmoid(gi[:, H : 2 * H] + gh[:, H : 2 * H])
    n0 = np.tanh(gi[:, 2 * H :] + r0 * gh[:, 2 * H :])
    h1 = (1.0 - z0) * n0 + z0 * h0  # [B, H]
    y0 = h1 @ W_fc.T + b_fc  # [B, OUT]

    W_comb = W_ih @ W_fc  # [3H, H]
    b_comb = b_ih + W_ih @ b_fc

    Wr = W_comb[:H] + W_hh[:H]
    Wz = W_comb[H : 2 * H] + W_hh[H : 2 * H]
    Wni = W_comb[2 * H :]
    Wnh = W_hh[2 * H :]
    Wbig = np.empty((4 * H, H), f32)
    for g in range(G):
        base = g * 512
        Wbig[base : base + 128] = Wr[g * 128 : (g + 1) * 128]
        Wbig[base + 128 : base + 256] = Wz[g * 128 : (g + 1) * 128]
        Wbig[base + 256 : base + 384] = Wni[g * 128 : (g + 1) * 128]
        Wbig[base + 384 : base + 512] = Wnh[g * 128 : (g + 1) * 128]
    w = _to_ktiles(Wbig.T, 4 * H).astype(bfloat16)  # [128, KT, 4H]
    wfc = _to_ktiles(np.ascontiguousarray(W_fc).T, OUT).astype(bfloat16)

    br = b_comb[:H] + b_hh[:H]
    bz = b_comb[H : 2 * H] + b_hh[H : 2 * H]
    bni = b_comb[2 * H :]
    bnh = b_hh[2 * H :]
    bias = np.stack(
        [b.reshape(G, 128) for b in (br, bz, bni, bnh)], axis=-1
    ).transpose(1, 0, 2)  # [128, G, 4]
    bias = np.ascontiguousarray(bias.astype(f32))
    bfc = np.ascontiguousarray(b_fc.reshape(MT, 128).T.astype(f32))

    in_maps = []
    for c in range(NCORES):
        h1c = _to_ktiles(
            np.ascontiguousarray(h1[c * BL : (c + 1) * BL].T), BL
        ).astype(bfloat16)
        in_maps.append(
            {
                "wsh": np.ascontiguousarray(w[:, :, c * MSH : (c + 1) * MSH]),
                "wfsh": np.ascontiguousarray(wfc[:, :, c * OSH : (c + 1) * OSH]),
                "h1": h1c,
                "bias": bias,
                "bfc": bfc,
            }
        )
    return in_maps, y0


def kernel(src, tgt, hidden, W_ih, W_hh, b_ih, b_hh, W_fc, b_fc, **_unused):
    from concourse import bass_utils

    if "nc" not in _cache:
        _cache["nc"] = _build_program()
    nc = _cache["nc"]

    in_maps, y0 = _prep_inputs(src, hidden, W_ih, W_hh, b_ih, b_hh, W_fc, b_fc)
    res = bass_utils.run_bass_kernel_spmd(nc, in_maps, core_ids=list(range(NCORES)))
    full = np.empty((T, B, OUT), np.float32)
    full[0] = y0
    for c in range(NCORES):
        yq = np.asarray(res.results[c]["out"]).astype(np.float32)  # [OUT, PAD]
        s = np.asarray(res.results[c]["s"])  # [OUT, N_CHUNKS*NB]
        yc = (yq.reshape(OUT, N_CHUNKS * NB, 512) * s[:, :, None]).reshape(
            OUT, PAD_COLS
        )[:, :N_COLS]
        full[1:, c * BL : (c + 1) * BL, :] = yc.reshape(OUT, T_DEV, BL).transpose(
            1, 2, 0
        )
    return full


# revision 13
# speedup vs baseline: 5.0659x; 1.0572x over previous
"""GRU decoder kernel for 8 trn2 NeuronCores — data-parallel, transfer-optimized.

Derivation: x_{t+1} = y_t = h_{t+1} @ W_fc.T + b_fc, so for t>=1 the
input-path matmul folds into the recurrence:
  gi_t = h_t @ (W_ih @ W_fc).T + (b_ih + W_ih @ b_fc).
r/z gates use gi+gh, so those rows of the folded matrix and W_hh are summed
host-side; the n-gate keeps gi_n / gh_n separate (r multiplies only gh_n).
Step 0 (needs x0 = src[0]) is computed exactly on the host in f32; the device
starts from h_1 and runs T-1 folded steps.

Sharding: data-parallel over batch (no per-step collectives). Core c owns
batch columns [32c, 32c+32). The time loop is a hardware For_i loop
(program ~1k instructions). h_t^T is DMA'd per step to a DRAM scratch; the
output projection runs as one batched matmul at the end.

The wall clock is dominated by the axon tunnel (~90 MB/s up, ~50 MB/s down),
so this version minimizes wire bytes:
  - weights are uploaded SHARDED (each core gets 1/8 of the folded W and of
    W_fc) and rebuilt on-device with one startup AllGather: 76 MB -> 10 MB.
  - the output is int8 with per-(row, 512-col-block) scales computed on
    device (absmax over each block row); the host dequantizes. This halves
    both the output download and the donated zero-buffer upload.
"""

import os
import sys

sys.path.insert(0, "/opt/trn_rl_repo")

import numpy as np

try:  # persistent XLA compile cache: skips ~0.2s/call of re-compile
    import jax

    jax.config.update("jax_compilation_cache_dir", "/tmp/jax_comp_cache")
    jax.config.update("jax_persistent_cache_min_entry_size_bytes", -1)
    jax.config.update("jax_persistent_cache_min_compile_time_secs", 0)
except Exception:
    pass

H = 1024
OUT = 768
B = 256
T = int(os.environ.get("GRU_T", "256"))
NCORES = 8
BL = B // NCORES  # 32 batch columns per core
KT = H // 128  # 8 k-tiles
G = 8  # hidden groups of 128 (= KT)
MT = OUT // 128  # 6 output m-tiles
T_DEV = T - 1  # steps computed on device
UNROLL = int(os.environ.get("GRU_UNROLL", "2"))  # steps per For_i iteration (even)
CHUNK = 2048  # fc columns per chunk (64 steps x 32 batch)
NB = CHUNK // 512
N_COLS = T_DEV * BL
N_CHUNKS = (N_COLS + CHUNK - 1) // CHUNK
PAD_COLS = N_CHUNKS * CHUNK
MSH = 4 * H // NCORES  # 512 folded-weight cols per core shard
OSH = OUT // NCORES  # 96 fc cols per core shard
QMAX = 126.5  # int8 headroom so the max element never wraps

_cache = {}


def _build_program():
    import concourse.mybir as mybir
    from concourse import bacc, tile
    from concourse.bass import ds, ts

    dt = mybir.dt
    AF = mybir.ActivationFunctionType
    OP = mybir.AluOpType
    RG = [list(range(NCORES))]

    nc = bacc.Bacc(num_devices=NCORES)

    # combined int8 weight shard: cols 0:MSH = folded-W shard, MSH: = W_fc shard
    wq_d = nc.dram_tensor("wq", [128, KT, MSH + OSH], dt.int8, kind="ExternalInput")
    sw_d = nc.dram_tensor("sw", [128, KT, 16], dt.float32, kind="ExternalInput")
    h1_d = nc.dram_tensor("h1", [128, KT, BL], dt.bfloat16, kind="ExternalInput")
    bias_d = nc.dram_tensor("bias", [128, G, 4], dt.float32, kind="ExternalInput")
    bfc_d = nc.dram_tensor("bfc", [128, MT], dt.float32, kind="ExternalInput")
    out_d = nc.dram_tensor("out", [OUT, PAD_COLS], dt.int8, kind="ExternalOutput")
    s_d = nc.dram_tensor(
        "s", [OUT, N_CHUNKS * NB], dt.float16, kind="ExternalOutput"
    )

    with tile.TileContext(nc) as tc:
        with (
            tc.tile_pool(name="wp", bufs=1) as wp,
            tc.tile_pool(name="hp", bufs=1) as hp,
            tc.tile_pool(name="ep", bufs=3) as ep,
            tc.tile_pool(name="pp", bufs=2, space="PSUM") as pp,
            tc.tile_pool(name="fp", bufs=2) as fp,
            tc.tile_pool(name="yp", bufs=2, space="PSUM") as yp,
            tc.tile_pool(name="dp", bufs=1, space="DRAM") as dp,
        ):
            # --- rebuild replicated weights from int8 shards with one
            # AllGather, then dequantize (per-(K-row, k-tile, shard-block)
            # scales) into bf16 SBUF tiles ---
            w_sb = wp.tile([128, KT, 4 * H], dt.bfloat16)
            wfc_sb = wp.tile([128, KT, OUT], dt.bfloat16)

            sw_sb = wp.tile([128, KT, 16], dt.float32)
            nc.sync.dma_start(sw_sb[:], sw_d[:])

            wst = fp.tile([128, KT, MSH + OSH], dt.int8, tag="wst")
            nc.sync.dma_start(wst[:], wq_d[:])
            cc_in = dp.tile([128, KT, MSH + OSH], dt.int8, tag="ccin")
            nc.sync.dma_start(cc_in[:], wst[:])
            cc_out = dp.tile([NCORES * 128, KT, MSH + OSH], dt.int8, tag="ccout")
            nc.gpsimd.collective_compute(
                "AllGather",
                mybir.AluOpType.bypass,
                replica_groups=RG,
                ins=[cc_in.opt()],
                outs=[cc_out.opt()],
            )
            for c in range(NCORES):
                stg = fp.tile([128, KT, MSH + OSH], dt.int8, tag="stg")
                nc.sync.dma_start(stg[:], cc_out[c * 128 : (c + 1) * 128])
                for k in range(KT):
                    nc.scalar.activation(
                        w_sb[:, k, c * MSH : (c + 1) * MSH],
                        stg[:, k, 0:MSH],
                        AF.Copy,
                        scale=sw_sb[:, k, c : c + 1],
                    )
                    nc.scalar.activation(
                        wfc_sb[:, k, c * OSH : (c + 1) * OSH],
                        stg[:, k, MSH : MSH + OSH],
                        AF.Copy,
                        scale=sw_sb[:, k, 8 + c : 8 + c + 1],
                    )

            bias_sb = wp.tile([128, G, 4], dt.float32)
            nc.sync.dma_start(bias_sb[:], bias_d[:])
            bfc_sb = wp.tile([128, MT], dt.float32)
            nc.sync.dma_start(bfc_sb[:], bfc_d[:])

            hA = hp.tile([128, KT, BL], dt.bfloat16, tag="hA")
            nc.sync.dma_start(hA[:], h1_d[:])
            hB = hp.tile([128, KT, BL], dt.bfloat16, tag="hB")

            H_d = dp.tile([128, KT, PAD_COLS], dt.bfloat16, tag="H")
            # zero H_d's padding columns so fc-block absmax sees no garbage
            if PAD_COLS > N_COLS:
                zt = fp.tile([128, KT, min(512, PAD_COLS - N_COLS)], dt.bfloat16,
                             tag="zt")
                nc.vector.memset(zt[:], 0.0)
                zoff = N_COLS
                while zoff < PAD_COLS:
                    zw = min(512, PAD_COLS - zoff)
                    nc.sync.dma_start(H_d[:, :, zoff : zoff + zw], zt[:, :, :zw])
                    zoff += zw

            def step(src, dst, t_iv):
                # h_{t+1}^T = gru_folded(h_t^T); also DMA to H_d[:, :, t*BL:]
                P = pp.tile([128, 4 * G * BL], dt.float32, tag="P")
                for g in range(G):
                    for m in range(4):
                        col = (g * 4 + m) * BL
                        lhs_col = (g * 4 + m) * 128
                        for k in range(KT):
                            nc.tensor.matmul(
                                P[:, col : col + BL],
                                w_sb[:, k, lhs_col : lhs_col + 128],
                                src[:, k, :],
                                start=(k == 0),
                                stop=(k == KT - 1),
                            )
                    base = g * 4 * BL
                    Pr = P[:, base : base + BL]
                    Pz = P[:, base + BL : base + 2 * BL]
                    Pni = P[:, base + 2 * BL : base + 3 * BL]
                    Pnh = P[:, base + 3 * BL : base + 4 * BL]
                    r = ep.tile([128, BL], dt.float32, tag="r")
                    nc.scalar.activation(r[:], Pr, AF.Sigmoid, bias=bias_sb[:, g, 0:1])
                    z = ep.tile([128, BL], dt.float32, tag="z")
                    nc.scalar.activation(z[:], Pz, AF.Sigmoid, bias=bias_sb[:, g, 1:2])
                    t2 = ep.tile([128, BL], dt.float32, tag="t2")
                    nc.vector.scalar_tensor_tensor(
                        t2[:], Pnh, bias_sb[:, g, 3:4], r[:], OP.add, OP.mult
                    )
                    t3 = ep.tile([128, BL], dt.float32, tag="t3")
                    nc.vector.tensor_add(t3[:], t2[:], Pni)
                    n = ep.tile([128, BL], dt.float32, tag="n")
                    nc.scalar.activation(n[:], t3[:], AF.Tanh, bias=bias_sb[:, g, 2:3])
                    d = ep.tile([128, BL], dt.float32, tag="d")
                    nc.vector.tensor_sub(d[:], src[:, g, :], n[:])
                    zd = ep.tile([128, BL], dt.float32, tag="zd")
                    nc.vector.tensor_mul(zd[:], z[:], d[:])
                    nc.vector.tensor_add(dst[:, g, :], n[:], zd[:])
                nc.sync.dma_start(H_d[:, :, ts(t_iv, BL)], dst[:])

            n_iters = T_DEV // UNROLL
            if n_iters > 0:
                with tc.For_i(0, n_iters * UNROLL, UNROLL) as iv:
                    for u in range(UNROLL):
                        sd = (hA, hB) if u % 2 == 0 else (hB, hA)
                        step(sd[0], sd[1], iv + u)
            for t in range(n_iters * UNROLL, T_DEV):
                sd = (hA, hB) if t % 2 == 0 else (hB, hA)
                step(sd[0], sd[1], t)

            # Batched output projection, quantized to int8 with per-row
            # per-512-col-block scales (absmax-based, computed on device).
            with tc.For_i(0, N_CHUNKS) as civ:
                Hc = fp.tile([128, KT, CHUNK], dt.bfloat16, tag="Hc")
                nc.sync.dma_start(Hc[:], H_d[:, :, ts(civ, CHUNK)])
                for m in range(MT):
                    for nb in range(NB):
                        Py = yp.tile([128, 512], dt.float32, tag="Py")
                        for k in range(KT):
                            nc.tensor.matmul(
                                Py[:],
                                wfc_sb[:, k, m * 128 : (m + 1) * 128],
                                Hc[:, k, nb * 512 : (nb + 1) * 512],
                                start=(k == 0),
                                stop=(k == KT - 1),
                            )
                        yf = fp.tile([128, 512], dt.float32, tag="yf")
                        nc.scalar.activation(
                            yf[:], Py[:], AF.Identity, bias=bfc_sb[:, m : m + 1]
                        )
                        sraw = ep.tile([128, 1], dt.float32, tag="sraw")
                        nc.vector.tensor_reduce(
                            sraw[:], yf[:], mybir.AxisListType.X, OP.max,
                            apply_absolute_value=True,
                        )
                        sq = ep.tile([128, 1], dt.float16, tag="sq")
                        nc.vector.tensor_scalar(
                            sq[:], sraw[:], 1.0 / QMAX, 1e-30, OP.mult, OP.add
                        )
                        inv = ep.tile([128, 1], dt.float32, tag="inv")
                        nc.vector.reciprocal(inv[:], sq[:])
                        yq = fp.tile([128, 512], dt.int8, tag="yq")
                        nc.scalar.activation(
                            yq[:], yf[:], AF.Identity, scale=inv[:, 0:1]
                        )
                        nc.sync.dma_start(
                            out_d[m * 128 : (m + 1) * 128, ts(civ * NB + nb, 512)],
                            yq[:],
                        )
                        nc.sync.dma_start(
                            s_d[m * 128 : (m + 1) * 128, ds(civ * NB + nb, 1)],
                            sq[:],
                        )

    nc.compile()
    return nc


def _sigmoid(v):
    return 1.0 / (1.0 + np.exp(-v))


def _to_ktiles(lhsT, m):  # [K, m] -> [128, K/128, m]
    k = lhsT.shape[0] // 128
    return np.ascontiguousarray(lhsT.reshape(k, 128, m).transpose(1, 0, 2))


def _prep_inputs(src, hidden, W_ih, W_hh, b_ih, b_hh, W_fc, b_fc):
    from ml_dtypes import bfloat16

    f32 = np.float32
    src = np.asarray(src, f32)
    hidden = np.asarray(hidden, f32)
    W_ih = np.asarray(W_ih, f32)
    W_hh = np.asarray(W_hh, f32)
    b_ih = np.asarray(b_ih, f32)
    b_hh = np.asarray(b_hh, f32)
    W_fc = np.asarray(W_fc, f32)
    b_fc = np.asarray(b_fc, f32)

    x0 = src[0]  # [B, OUT]
    h0 = hidden[0]  # [B, H]

    # exact f32 step 0 on host
    gi = x0 @ W_ih.T + b_ih
    gh = h0 @ W_hh.T + b_hh
    r0 = _sigmoid(gi[:, :H] + gh[:, :H])
    z0 = _sigmoid(gi[:, H : 2 * H] + gh[:, H : 2 * H])
    n0 = np.tanh(gi[:, 2 * H :] + r0 * gh[:, 2 * H :])
    h1 = (1.0 - z0) * n0 + z0 * h0  # [B, H]
    y0 = h1 @ W_fc.T + b_fc  # [B, OUT]

    W_comb = W_ih @ W_fc  # [3H, H]
    b_comb = b_ih + W_ih @ b_fc

    Wr = W_comb[:H] + W_hh[:H]
    Wz = W_comb[H : 2 * H] + W_hh[H : 2 * H]
    Wni = W_comb[2 * H :]
    Wnh = W_hh[2 * H :]
    Wbig = np.empty((4 * H, H), f32)
    for g in range(G):
        base = g * 512
        Wbig[base : base + 128] = Wr[g * 128 : (g + 1) * 128]
        Wbig[base + 128 : base + 256] = Wz[g * 128 : (g + 1) * 128]
        Wbig[base + 256 : base + 384] = Wni[g * 128 : (g + 1) * 128]
        Wbig[base + 384 : base + 512] = Wnh[g * 128 : (g + 1) * 128]
    # int8 quantization of the k-tiled weights, per (K-row, k-tile,
    # core-shard block): absmax/127 scales, exact rint in [-127, 127].
    w = _to_ktiles(Wbig.T, 4 * H)  # [128, KT, 4H] f32
    wfc = _to_ktiles(np.ascontiguousarray(W_fc).T, OUT)
    am_w = np.abs(w.reshape(128, KT, NCORES, MSH)).max(axis=-1)  # [128,KT,8]
    sw_w = am_w / 127.0 + 1e-30
    wq_w = np.rint(w.reshape(128, KT, NCORES, MSH) / sw_w[..., None]).astype(
        np.int8
    )
    am_f = np.abs(wfc.reshape(128, KT, NCORES, OSH)).max(axis=-1)
    sw_f = am_f / 127.0 + 1e-30
    wq_f = np.rint(wfc.reshape(128, KT, NCORES, OSH) / sw_f[..., None]).astype(
        np.int8
    )
    sw = np.ascontiguousarray(
        np.concatenate([sw_w, sw_f], axis=2).astype(f32)
    )  # [128, KT, 16]

    br = b_comb[:H] + b_hh[:H]
    bz = b_comb[H : 2 * H] + b_hh[H : 2 * H]
    bni = b_comb[2 * H :]
    bnh = b_hh[2 * H :]
    bias = np.stack(
        [b.reshape(G, 128) for b in (br, bz, bni, bnh)], axis=-1
    ).transpose(1, 0, 2)  # [128, G, 4]
    bias = np.ascontiguousarray(bias.astype(f32))
    bfc = np.ascontiguousarray(b_fc.reshape(MT, 128).T.astype(f32))

    in_maps = []
    for c in range(NCORES):
        h1c = _to_ktiles(
            np.ascontiguousarray(h1[c * BL : (c + 1) * BL].T), BL
        ).astype(bfloat16)
        wq = np.concatenate([wq_w[:, :, c, :], wq_f[:, :, c, :]], axis=2)
        in_maps.append(
            {
                "wq": np.ascontiguousarray(wq),
                "sw": sw,
                "h1": h1c,
                "bias": bias,
                "bfc": bfc,
            }
        )
    return in_maps, y0


def kernel(src, tgt, hidden, W_ih, W_hh, b_ih, b_hh, W_fc, b_fc, **_unused):
    from concourse import bass_utils

    if "nc" not in _cache:
        _cache["nc"] = _build_program()
    nc = _cache["nc"]

    in_maps, y0 = _prep_inputs(src, hidden, W_ih, W_hh, b_ih, b_hh, W_fc, b_fc)
    res = bass_utils.run_bass_kernel_spmd(nc, in_maps, core_ids=list(range(NCORES)))
    full = np.empty((T, B, OUT), np.float32)
    full[0] = y0
    for c in range(NCORES):
        yq = np.asarray(res.results[c]["out"]).astype(np.float32)  # [OUT, PAD]
        s = np.asarray(res.results[c]["s"]).astype(np.float32)  # [OUT, NCH*NB]
        yc = (yq.reshape(OUT, N_CHUNKS * NB, 512) * s[:, :, None]).reshape(
            OUT, PAD_COLS
        )[:, :N_COLS]
        full[1:, c * BL : (c + 1) * BL, :] = yc.reshape(OUT, T_DEV, BL).transpose(
            1, 2, 0
        )
    return full
